# revision 35
# baseline (speedup 1.0000x reference)
"""Trainium2 Bass kernel for:
    tgt_norm = tgt / ||tgt||_2 (rows)
    sim      = tgt_norm @ tgt_norm.T          (per batch, NxN)
    out      = tanh(sim) @ tgt                (per batch, NxD)

Sharding: data-parallel over batch B=8, one batch per NeuronCore.
Per-core fused flash-style kernel; sim is never materialized in HBM.
Symmetric mode computes only the lower-triangular key panels of sim,
spills tanh tiles to DRAM, and replays them transposed (DMA-xbar) as
the mirror-block weights in a second pass.

Self-contained: only needs the concourse tree staged on the machine.
"""

import sys

for _p in ("/opt/trn_rl_repo",):
    if _p not in sys.path:
        sys.path.append(_p)

import numpy as np

import concourse.bacc as bacc
import concourse.mybir as mybir
import concourse.tile as tile
from concourse.bass_utils import run_bass_kernel_spmd

P = 128  # partitions

F32 = mybir.dt.float32
BF16 = mybir.dt.bfloat16
AF = mybir.ActivationFunctionType


def build_kernel(N=4096, D=512, QB=512, symmetric=True):
    """One NeuronCore program: tgt [N, D] f32 -> out [N, D] f32."""
    NT = N // P   # key tiles (128 rows each)
    DC = D // P   # contraction chunks of the feature dim
    NQ = N // QB  # query panels
    XQ = QB // P  # 128-row sub-blocks per query panel

    nc = bacc.Bacc(debug=False)
    tgt = nc.dram_tensor("tgt", [N, D], F32, kind="ExternalInput")
    out = nc.dram_tensor("out", [N, D], F32, kind="ExternalOutput")

    with tile.TileContext(nc) as tc:
        with (
            tc.tile_pool(name="persist", bufs=1) as pb,
            tc.tile_pool(name="sq", bufs=3) as sqp,
            tc.tile_pool(name="stage", bufs=4) as stp,
            tc.tile_pool(name="tanh", bufs=4) as thp,
            tc.tile_pool(name="osb", bufs=3) as osp,
            tc.tile_pool(name="mirA", bufs=6) as mupA,
            tc.tile_pool(name="mirB", bufs=6) as mupB,
            tc.tile_pool(name="dram", bufs=1, space="DRAM") as dp,
            tc.tile_pool(name="ps_out", bufs=4, space="PSUM") as pso,
            tc.tile_pool(name="ps_sim", bufs=4, space="PSUM") as pss,
        ):
            # ---------------- phase 1: load, norms, casts, transpose -------
            tgtb = pb.tile([P, NT * D], BF16)     # tgt, N-major bf16
            tnT = pb.tile([P, DC * N], BF16)      # normalized tgt, D-major

            tnT_v = tnT[:].rearrange("p (c n) -> p c n", c=DC)
            tgtb_v = tgtb[:].rearrange("p (t d) -> p t d", t=NT)

            if symmetric:
                # tanh spill: spill_d[a, r, k*P+c] = tanh_simT block
                # [keys tile a (rows r), queries tile k (cols c)]
                spill_d = dp.tile([NT, P, N], BF16)
                # partial outputs live in SBUF (panels 0..NQ-2)
                oacc = [pb.tile([P, D], F32, name=f"oacc{t}", tag=f"oacc{t}")
                        for t in range(NT - XQ)]

            # grouped loads: one DMA trigger per XQ tiles; first 4 upfront,
            # later ones re-emitted as slots free (avoids ACT head-of-line)
            GL = XQ
            NG = NT // GL
            lds = [None] * NG

            def emit_load(g):
                ld = sqp.tile([P, GL * D], F32, name=f"ldg{g}", tag="ldg")
                nc.scalar.dma_start(
                    ld[:].rearrange("p (t d) -> p t d", t=GL),
                    tgt[g * GL * P:(g + 1) * GL * P, :]
                    .rearrange("(t p) d -> p t d", p=P))
                lds[g] = ld

            for g in range(min(4, NG)):
                emit_load(g)

            def group_chain(g):
                ld = lds[g]
                ss = stp.tile([P, GL], F32, name="ss", tag="ss")
                for i in range(GL):
                    sl = ld[:, i * D:(i + 1) * D]
                    sq = sqp.tile([P, D], F32, name="sq", tag="sq")
                    if i % 2 == 0:
                        nc.scalar.activation(sq[:], sl, AF.Square,
                                             accum_out=ss[:, i:i + 1])
                    else:
                        nc.vector.tensor_mul(sq[:], sl, sl)
                        nc.vector.tensor_reduce(ss[:, i:i + 1], sq[:],
                                                axis=mybir.AxisListType.X,
                                                op=mybir.AluOpType.add)
                nn_ = stp.tile([P, GL], F32, name="nn", tag="nn")
                nc.scalar.sqrt(nn_[:], ss[:])
                iv = stp.tile([P, GL], F32, name="iv", tag="iv")
                nc.vector.reciprocal(iv[:], nn_[:])
                for i in range(GL):
                    j = g * GL + i
                    sl = ld[:, i * D:(i + 1) * D]
                    stg = stp.tile([P, D], BF16, name="stg", tag="stg")
                    nc.vector.tensor_scalar_mul(stg[:], sl, iv[:, i:i + 1])
                    nc.sync.dma_start_transpose(
                        tnT_v[:, :, j * P:(j + 1) * P], stg[:])
                    nc.vector.tensor_copy(tgtb[:, j * D:(j + 1) * D], sl)
                if g + 4 < NG:
                    emit_load(g + 4)

            emitted = [0]

            def ensure_tiles(n):
                while emitted[0] * GL <= min(n, NT - 1):
                    group_chain(emitted[0])
                    emitted[0] += 1

            # ---------------- pass 1: fused sim -> tanh -> out -------------
            for qi in range(NQ):
                kmax = (qi + 1) * XQ if symmetric else NT
                # emit chains two panels ahead so tile production streams
                # instead of locking step with each panel's tanh burst
                ensure_tiles(kmax - 1 + 2 * XQ if symmetric else XQ - 1)
                out_ps = [pso.tile([P, D], F32, tag="ops", name=f"ops{x}")
                          for x in range(XQ)]

                def out_mms(kj, th):
                    for x in range(XQ):
                        nc.tensor.matmul(
                            out_ps[x][:],
                            th[:, x * P:(x + 1) * P],
                            tgtb_v[:, kj, :],
                            start=(kj == 0), stop=(kj == kmax - 1),
                        )
                    if symmetric and kj < qi * XQ:
                        nc.scalar.dma_start(
                            spill_d[kj, :, qi * QB:(qi + 1) * QB], th[:])

                prev = None
                prev_kj = None
                for kj in range(kmax):
                    if not symmetric and qi == 0:
                        ensure_tiles(kj + 3)
                    sim_ps = pss.tile([P, QB], F32)
                    for c in range(DC):
                        nc.tensor.matmul(
                            sim_ps[:],
                            tnT_v[:, c, kj * P:(kj + 1) * P],
                            tnT_v[:, c, qi * QB:(qi + 1) * QB],
                            start=(c == 0), stop=(c == DC - 1),
                        )
                    if prev is not None:
                        out_mms(prev_kj, prev)
                    th = thp.tile([P, QB], BF16)
                    nc.scalar.activation(th[:], sim_ps[:], AF.Tanh)
                    prev, prev_kj = th, kj
                out_mms(prev_kj, prev)

                last_panel = qi == NQ - 1
                for x in range(XQ):
                    t = qi * XQ + x
                    if symmetric and not last_panel:
                        nc.vector.tensor_copy(oacc[t][:], out_ps[x][:])
                    else:
                        ob = osp.tile([P, D], F32, name="ob", tag="ob")
                        nc.vector.tensor_copy(ob[:], out_ps[x][:])
                        nc.gpsimd.dma_start(out[t * P:(t + 1) * P, :], ob[:])

            # ---------------- pass 2: mirror contributions -----------------
            if symmetric:
                # hard fence: pass-2 xbar reads must not race pass-1 spills
                tc.strict_bb_all_engine_barrier()
                GK = 8  # key tiles per transposing read
                for bi in range(NQ - 1):
                    k0 = (bi + 1) * XQ
                    # alternate PSUM pools so consecutive panels never wait
                    # on each other's bank release
                    pool2 = pso if bi % 2 == 0 else pss
                    tag2 = "ops" if bi % 2 == 0 else "sim_ps"
                    out_ps2 = [pool2.tile([P, D], F32, name=f"ops2_{x}",
                                          tag=tag2) for x in range(XQ)]
                    mup = mupA if bi % 2 == 0 else mupB
                    for x in range(XQ):
                        b = bi * XQ + x
                        for g0 in range(k0, NT, GK):
                            m = min(GK, NT - g0)
                            u = mup.tile([P, GK * P], BF16, name="u", tag="u")
                            u_v = u[:].rearrange("p (w y) -> p w y", w=GK)
                            nc.sync.dma_start_transpose(
                                u_v[:, :m, :],
                                spill_d[b, :, g0 * P:(g0 + m) * P])
                            for w in range(m):
                                k = g0 + w
                                nc.tensor.matmul(
                                    out_ps2[x][:], u_v[:, w, :],
                                    tgtb_v[:, k, :],
                                    start=(k == k0), stop=(k == NT - 1),
                                )
                        ob = osp.tile([P, D], F32, name="ob2", tag="ob2")
                        nc.vector.tensor_add(ob[:], out_ps2[x][:], oacc[b][:])
                        nc.scalar.dma_start(out[b * P:(b + 1) * P, :], ob[:])

    nc.compile()
    return nc


_cache = {}


def _get_nc(N, D):
    key = (N, D)
    if key not in _cache:
        _cache[key] = build_kernel(N, D)
    return _cache[key]


def _run(tgt, trace=False):
    """tgt: [B, N, D] f32. Returns (out [B, N, D] f32, exec_time_ns|None)."""
    tgt = np.ascontiguousarray(np.asarray(tgt, dtype=np.float32))
    B, N, D = tgt.shape
    nc = _get_nc(N, D)
    in_maps = [{"tgt": tgt[b]} for b in range(B)]
    res = run_bass_kernel_spmd(nc, in_maps, core_ids=list(range(B)), trace=trace)
    outp = np.stack([res.results[b]["out"] for b in range(B)], axis=0)
    return outp.astype(np.float32), res.exec_time_ns


def kernel(tgt, query_pos=None, objects_num=None, **_unused):
    out, _ = _run(tgt, trace=False)
    return out


# revision 38
# speedup vs baseline: 1.0164x; 1.0164x over previous
"""Trainium2 Bass kernel for:
    tgt_norm = tgt / ||tgt||_2 (rows)
    sim      = tgt_norm @ tgt_norm.T          (per batch, NxN)
    out      = tanh(sim) @ tgt                (per batch, NxD)

Sharding: data-parallel over batch B=8, one batch per NeuronCore.
Per-core fused flash-style kernel; sim is never materialized in HBM.
Symmetric mode computes only the lower-triangular key panels of sim,
spills tanh tiles to DRAM, and replays them transposed (DMA-xbar) as
the mirror-block weights in a second pass.

Self-contained: only needs the concourse tree staged on the machine.
"""

import sys

for _p in ("/opt/trn_rl_repo",):
    if _p not in sys.path:
        sys.path.append(_p)

import numpy as np

import concourse.bacc as bacc
import concourse.mybir as mybir
import concourse.tile as tile
from concourse.bass_utils import run_bass_kernel_spmd

P = 128  # partitions

F32 = mybir.dt.float32
BF16 = mybir.dt.bfloat16
AF = mybir.ActivationFunctionType


def build_kernel(N=4096, D=512, QB=512, symmetric=True):
    """One NeuronCore program: tgt [N, D] f32 -> out [N, D] f32."""
    NT = N // P   # key tiles (128 rows each)
    DC = D // P   # contraction chunks of the feature dim
    NQ = N // QB  # query panels
    XQ = QB // P  # 128-row sub-blocks per query panel

    nc = bacc.Bacc(debug=False)
    tgt = nc.dram_tensor("tgt", [N, D], F32, kind="ExternalInput")
    out = nc.dram_tensor("out", [N, D], F32, kind="ExternalOutput")

    with tile.TileContext(nc) as tc:
        with (
            tc.tile_pool(name="persist", bufs=1) as pb,
            tc.tile_pool(name="sq", bufs=2) as sqp,
            tc.tile_pool(name="stage", bufs=3) as stp,
            tc.tile_pool(name="tanh", bufs=2) as thp,
            tc.tile_pool(name="osb", bufs=2) as osp,
            tc.tile_pool(name="mirA", bufs=3) as mupA,
            tc.tile_pool(name="mirB", bufs=3) as mupB,
            tc.tile_pool(name="dram", bufs=1, space="DRAM") as dp,
            tc.tile_pool(name="ps_out", bufs=4, space="PSUM") as pso,
            tc.tile_pool(name="ps_sim", bufs=4, space="PSUM") as pss,
        ):
            # ---------------- phase 1: load, norms, casts, transpose -------
            tgtb = pb.tile([P, NT * D], BF16)     # tgt, N-major bf16
            tnT = pb.tile([P, DC * N], BF16)      # normalized tgt, D-major

            tnT_v = tnT[:].rearrange("p (c n) -> p c n", c=DC)
            tgtb_v = tgtb[:].rearrange("p (t d) -> p t d", t=NT)

            if symmetric:
                # tanh spill: spill_d[a, r, k*P+c] = tanh_simT block
                # [keys tile a (rows r), queries tile k (cols c)]
                spill_d = dp.tile([NT, P, N], BF16)
                # partial outputs live in SBUF (panels 0..NQ-2)
                oacc = [pb.tile([P, D], F32, name=f"oacc{t}", tag=f"oacc{t}")
                        for t in range(NT - XQ)]

            # grouped loads: one DMA trigger per XQ tiles; first 4 upfront,
            # later ones re-emitted as slots free (avoids ACT head-of-line)
            GL = XQ
            NG = NT // GL
            lds = [None] * NG

            def emit_load(g):
                ld = sqp.tile([P, GL * D], F32, name=f"ldg{g}", tag="ldg", bufs=3)
                nc.scalar.dma_start(
                    ld[:].rearrange("p (t d) -> p t d", t=GL),
                    tgt[g * GL * P:(g + 1) * GL * P, :]
                    .rearrange("(t p) d -> p t d", p=P))
                lds[g] = ld

            for g in range(min(3, NG)):
                emit_load(g)

            def group_chain(g):
                ld = lds[g]
                ss = stp.tile([P, GL], F32, name="ss", tag="ss")
                for i in range(GL):
                    sl = ld[:, i * D:(i + 1) * D]
                    sq = sqp.tile([P, D], F32, name="sq", tag="sq")
                    if i % 2 == 0:
                        nc.scalar.activation(sq[:], sl, AF.Square,
                                             accum_out=ss[:, i:i + 1])
                    else:
                        nc.vector.tensor_mul(sq[:], sl, sl)
                        nc.vector.tensor_reduce(ss[:, i:i + 1], sq[:],
                                                axis=mybir.AxisListType.X,
                                                op=mybir.AluOpType.add)
                nn_ = stp.tile([P, GL], F32, name="nn", tag="nn")
                nc.scalar.sqrt(nn_[:], ss[:])
                iv = stp.tile([P, GL], F32, name="iv", tag="iv")
                nc.vector.reciprocal(iv[:], nn_[:])
                for i in range(GL):
                    j = g * GL + i
                    sl = ld[:, i * D:(i + 1) * D]
                    stg = stp.tile([P, D], BF16, name="stg", tag="stg")
                    nc.vector.tensor_scalar_mul(stg[:], sl, iv[:, i:i + 1])
                    nc.sync.dma_start_transpose(
                        tnT_v[:, :, j * P:(j + 1) * P], stg[:])
                    nc.vector.tensor_copy(tgtb[:, j * D:(j + 1) * D], sl)
                if g + 3 < NG:
                    emit_load(g + 3)

            emitted = [0]

            def ensure_tiles(n):
                while emitted[0] * GL <= min(n, NT - 1):
                    group_chain(emitted[0])
                    emitted[0] += 1

            # ---------------- pass 1: fused sim -> tanh -> out -------------
            for qi in range(NQ):
                kmax = (qi + 1) * XQ if symmetric else NT
                # emit chains two panels ahead so tile production streams
                # instead of locking step with each panel's tanh burst
                ensure_tiles(kmax - 1 + 2 * XQ if symmetric else XQ - 1)
                out_ps = [pso.tile([P, D], F32, tag="ops", name=f"ops{x}")
                          for x in range(XQ)]

                def out_mms(kj, th):
                    for x in range(XQ):
                        nc.tensor.matmul(
                            out_ps[x][:],
                            th[:, x * P:(x + 1) * P],
                            tgtb_v[:, kj, :],
                            start=(kj == 0), stop=(kj == kmax - 1),
                        )
                    if symmetric and kj < qi * XQ:
                        nc.scalar.dma_start(
                            spill_d[kj, :, qi * QB:(qi + 1) * QB], th[:])

                prev = None
                prev_kj = None
                for kj in range(kmax):
                    if not symmetric and qi == 0:
                        ensure_tiles(kj + 3)
                    sim_ps = pss.tile([P, QB], F32)
                    for c in range(DC):
                        nc.tensor.matmul(
                            sim_ps[:],
                            tnT_v[:, c, kj * P:(kj + 1) * P],
                            tnT_v[:, c, qi * QB:(qi + 1) * QB],
                            start=(c == 0), stop=(c == DC - 1),
                        )
                    if prev is not None:
                        out_mms(prev_kj, prev)
                    th = thp.tile([P, QB], BF16)
                    nc.scalar.activation(th[:], sim_ps[:], AF.Tanh)
                    prev, prev_kj = th, kj
                out_mms(prev_kj, prev)

                last_panel = qi == NQ - 1
                for x in range(XQ):
                    t = qi * XQ + x
                    if symmetric and not last_panel:
                        nc.vector.tensor_copy(oacc[t][:], out_ps[x][:])
                    else:
                        ob = osp.tile([P, D], F32, name="ob", tag="ob")
                        nc.vector.tensor_copy(ob[:], out_ps[x][:])
                        nc.gpsimd.dma_start(out[t * P:(t + 1) * P, :], ob[:])

            # ---------------- pass 2: mirror contributions -----------------
            if symmetric:
                # hard fence: pass-2 xbar reads must not race pass-1 spills
                tc.strict_bb_all_engine_barrier()
                GK = 16  # key tiles per transposing read
                for bi in range(NQ - 1):
                    k0 = (bi + 1) * XQ
                    # alternate PSUM pools so consecutive panels never wait
                    # on each other's bank release
                    pool2 = pso if bi % 2 == 0 else pss
                    tag2 = "ops" if bi % 2 == 0 else "sim_ps"
                    out_ps2 = [pool2.tile([P, D], F32, name=f"ops2_{x}",
                                          tag=tag2) for x in range(XQ)]
                    mup = mupA if bi % 2 == 0 else mupB
                    groups = list(range(k0, NT, GK))
                    for gi, g0 in enumerate(groups):
                        m = min(GK, NT - g0)
                        # all four x-transposes of the group first, so the
                        # mirror weights prefetch ahead of the matmuls
                        us = []
                        for x in range(XQ):
                            b = bi * XQ + x
                            u = mup.tile([P, GK * P], BF16, name="u", tag="u")
                            u_v = u[:].rearrange("p (w y) -> p w y", w=GK)
                            nc.sync.dma_start_transpose(
                                u_v[:, :m, :],
                                spill_d[b, :, g0 * P:(g0 + m) * P])
                            us.append(u_v)
                        for x in range(XQ):
                            for w in range(m):
                                k = g0 + w
                                nc.tensor.matmul(
                                    out_ps2[x][:], us[x][:, w, :],
                                    tgtb_v[:, k, :],
                                    start=(k == k0), stop=(k == NT - 1),
                                )
                    for x in range(XQ):
                        b = bi * XQ + x
                        ob = osp.tile([P, D], F32, name="ob2", tag="ob2")
                        nc.vector.tensor_add(ob[:], out_ps2[x][:], oacc[b][:])
                        nc.scalar.dma_start(out[b * P:(b + 1) * P, :], ob[:])

    nc.compile()
    return nc


_cache = {}


def _get_nc(N, D):
    key = (N, D)
    if key not in _cache:
        _cache[key] = build_kernel(N, D)
    return _cache[key]


def _run(tgt, trace=False):
    """tgt: [B, N, D] f32. Returns (out [B, N, D] f32, exec_time_ns|None)."""
    tgt = np.ascontiguousarray(np.asarray(tgt, dtype=np.float32))
    B, N, D = tgt.shape
    nc = _get_nc(N, D)
    in_maps = [{"tgt": tgt[b]} for b in range(B)]
    res = run_bass_kernel_spmd(nc, in_maps, core_ids=list(range(B)), trace=trace)
    outp = np.stack([res.results[b]["out"] for b in range(B)], axis=0)
    return outp.astype(np.float32), res.exec_time_ns


def kernel(tgt, query_pos=None, objects_num=None, **_unused):
    out, _ = _run(tgt, trace=False)
    return out


# revision 39
# speedup vs baseline: 1.1275x; 1.1094x over previous
"""Trainium2 Bass kernel for:
    tgt_norm = tgt / ||tgt||_2 (rows)
    sim      = tgt_norm @ tgt_norm.T          (per batch, NxN)
    out      = tanh(sim) @ tgt                (per batch, NxD)

Sharding: data-parallel over batch B=8, one batch per NeuronCore.
Per-core fused flash-style kernel; sim is never materialized in HBM.
Symmetric mode computes only the lower-triangular key panels of sim,
spills tanh tiles to DRAM, and replays them transposed (DMA-xbar) as
the mirror-block weights in a second pass.

Self-contained: only needs the concourse tree staged on the machine.
"""

import sys

for _p in ("/opt/trn_rl_repo",):
    if _p not in sys.path:
        sys.path.append(_p)

import numpy as np

import concourse.bacc as bacc
import concourse.mybir as mybir
import concourse.tile as tile
from concourse.bass_utils import run_bass_kernel_spmd

P = 128  # partitions

F32 = mybir.dt.float32
BF16 = mybir.dt.bfloat16
AF = mybir.ActivationFunctionType


def build_kernel(N=4096, D=512, QB=512, symmetric=True):
    """One NeuronCore program: tgt [N, D] f32 -> out [N, D] f32."""
    NT = N // P   # key tiles (128 rows each)
    DC = D // P   # contraction chunks of the feature dim
    NQ = N // QB  # query panels
    XQ = QB // P  # 128-row sub-blocks per query panel

    nc = bacc.Bacc(debug=False)
    tgt = nc.dram_tensor("tgt", [N, D], F32, kind="ExternalInput")
    out = nc.dram_tensor("out", [N, D], F32, kind="ExternalOutput")

    with tile.TileContext(nc) as tc:
        with (
            tc.tile_pool(name="persist", bufs=1) as pb,
            tc.tile_pool(name="sq", bufs=2) as sqp,
            tc.tile_pool(name="stage", bufs=3) as stp,
            tc.tile_pool(name="tanh", bufs=3) as thp,
            tc.tile_pool(name="osb", bufs=2) as osp,
            tc.tile_pool(name="mirA", bufs=4) as mupA,
            tc.tile_pool(name="mirB", bufs=4) as mupB,
            tc.tile_pool(name="dram", bufs=1, space="DRAM") as dp,
            tc.tile_pool(name="ps_out", bufs=4, space="PSUM") as pso,
            tc.tile_pool(name="ps_sim", bufs=4, space="PSUM") as pss,
        ):
            # ---------------- phase 1: load, norms, casts, transpose -------
            tgtb = pb.tile([P, NT * D], BF16)     # tgt, N-major bf16
            tnT = pb.tile([P, DC * N], BF16)      # normalized tgt, D-major

            tnT_v = tnT[:].rearrange("p (c n) -> p c n", c=DC)
            tgtb_v = tgtb[:].rearrange("p (t d) -> p t d", t=NT)

            if symmetric:
                # tanh spill: spill_d[a, r, k*P+c] = tanh_simT block
                # [keys tile a (rows r), queries tile k (cols c)]
                spill_d = dp.tile([NT, P, N], BF16)
                # partial outputs live in SBUF (panels 0..NQ-2)
                oacc = [pb.tile([P, D], BF16, name=f"oacc{t}", tag=f"oacc{t}")
                        for t in range(NT - XQ)]

            # grouped loads: one DMA trigger per XQ tiles; first 4 upfront,
            # later ones re-emitted as slots free (avoids ACT head-of-line)
            GL = XQ
            NG = NT // GL
            lds = [None] * NG

            def emit_load(g):
                ld = sqp.tile([P, GL * D], F32, name=f"ldg{g}", tag="ldg", bufs=3)
                nc.scalar.dma_start(
                    ld[:].rearrange("p (t d) -> p t d", t=GL),
                    tgt[g * GL * P:(g + 1) * GL * P, :]
                    .rearrange("(t p) d -> p t d", p=P))
                lds[g] = ld

            for g in range(min(3, NG)):
                emit_load(g)

            def group_chain(g):
                ld = lds[g]
                ss = stp.tile([P, GL], F32, name="ss", tag="ss")
                for i in range(GL):
                    sl = ld[:, i * D:(i + 1) * D]
                    sq = sqp.tile([P, D], F32, name="sq", tag="sq")
                    if i % 2 == 0:
                        nc.scalar.activation(sq[:], sl, AF.Square,
                                             accum_out=ss[:, i:i + 1])
                    else:
                        nc.vector.tensor_mul(sq[:], sl, sl)
                        nc.vector.tensor_reduce(ss[:, i:i + 1], sq[:],
                                                axis=mybir.AxisListType.X,
                                                op=mybir.AluOpType.add)
                nn_ = stp.tile([P, GL], F32, name="nn", tag="nn")
                nc.scalar.sqrt(nn_[:], ss[:])
                iv = stp.tile([P, GL], F32, name="iv", tag="iv")
                nc.vector.reciprocal(iv[:], nn_[:])
                for i in range(GL):
                    j = g * GL + i
                    sl = ld[:, i * D:(i + 1) * D]
                    stg = stp.tile([P, D], BF16, name="stg", tag="stg")
                    nc.vector.tensor_scalar_mul(stg[:], sl, iv[:, i:i + 1])
                    nc.sync.dma_start_transpose(
                        tnT_v[:, :, j * P:(j + 1) * P], stg[:])
                    nc.vector.tensor_copy(tgtb[:, j * D:(j + 1) * D], sl)
                if g + 3 < NG:
                    emit_load(g + 3)

            emitted = [0]

            def ensure_tiles(n):
                while emitted[0] * GL <= min(n, NT - 1):
                    group_chain(emitted[0])
                    emitted[0] += 1

            # ---------------- pass 1: fused sim -> tanh -> out -------------
            for qi in range(NQ):
                kmax = (qi + 1) * XQ if symmetric else NT
                # emit chains two panels ahead so tile production streams
                # instead of locking step with each panel's tanh burst
                ensure_tiles(kmax - 1 + 2 * XQ if symmetric else XQ - 1)
                out_ps = [pso.tile([P, D], F32, tag="ops", name=f"ops{x}")
                          for x in range(XQ)]

                def out_mms(kj, th):
                    for x in range(XQ):
                        nc.tensor.matmul(
                            out_ps[x][:],
                            th[:, x * P:(x + 1) * P],
                            tgtb_v[:, kj, :],
                            start=(kj == 0), stop=(kj == kmax - 1),
                        )
                    if symmetric and kj < qi * XQ:
                        nc.scalar.dma_start(
                            spill_d[kj, :, qi * QB:(qi + 1) * QB], th[:])

                prev = None
                prev_kj = None
                for kj in range(kmax):
                    if not symmetric and qi == 0:
                        ensure_tiles(kj + 3)
                    sim_ps = pss.tile([P, QB], F32)
                    for c in range(DC):
                        nc.tensor.matmul(
                            sim_ps[:],
                            tnT_v[:, c, kj * P:(kj + 1) * P],
                            tnT_v[:, c, qi * QB:(qi + 1) * QB],
                            start=(c == 0), stop=(c == DC - 1),
                        )
                    if prev is not None:
                        out_mms(prev_kj, prev)
                    th = thp.tile([P, QB], BF16)
                    nc.scalar.activation(th[:], sim_ps[:], AF.Tanh)
                    prev, prev_kj = th, kj
                out_mms(prev_kj, prev)

                last_panel = qi == NQ - 1
                for x in range(XQ):
                    t = qi * XQ + x
                    if symmetric and not last_panel:
                        nc.vector.tensor_copy(oacc[t][:], out_ps[x][:])
                    else:
                        ob = osp.tile([P, D], F32, name="ob", tag="ob")
                        nc.vector.tensor_copy(ob[:], out_ps[x][:])
                        nc.gpsimd.dma_start(out[t * P:(t + 1) * P, :], ob[:])

            # ---------------- pass 2: mirror contributions -----------------
            if symmetric:
                # hard fence: pass-2 xbar reads must not race pass-1 spills
                tc.strict_bb_all_engine_barrier()
                GK = 16  # key tiles per transposing read
                for bi in range(NQ - 1):
                    k0 = (bi + 1) * XQ
                    # alternate PSUM pools so consecutive panels never wait
                    # on each other's bank release
                    pool2 = pso if bi % 2 == 0 else pss
                    tag2 = "ops" if bi % 2 == 0 else "sim_ps"
                    out_ps2 = [pool2.tile([P, D], F32, name=f"ops2_{x}",
                                          tag=tag2) for x in range(XQ)]
                    mup = mupA if bi % 2 == 0 else mupB
                    groups = list(range(k0, NT, GK))
                    for gi, g0 in enumerate(groups):
                        m = min(GK, NT - g0)
                        # all four x-transposes of the group first, so the
                        # mirror weights prefetch ahead of the matmuls
                        us = []
                        for x in range(XQ):
                            b = bi * XQ + x
                            u = mup.tile([P, GK * P], BF16, name="u", tag="u")
                            u_v = u[:].rearrange("p (w y) -> p w y", w=GK)
                            nc.sync.dma_start_transpose(
                                u_v[:, :m, :],
                                spill_d[b, :, g0 * P:(g0 + m) * P])
                            us.append(u_v)
                        for x in range(XQ):
                            for w in range(m):
                                k = g0 + w
                                nc.tensor.matmul(
                                    out_ps2[x][:], us[x][:, w, :],
                                    tgtb_v[:, k, :],
                                    start=(k == k0), stop=(k == NT - 1),
                                )
                    for x in range(XQ):
                        b = bi * XQ + x
                        ob = osp.tile([P, D], F32, name="ob2", tag="ob2", bufs=4)
                        nc.vector.tensor_add(ob[:], out_ps2[x][:], oacc[b][:])
                        nc.scalar.dma_start(out[b * P:(b + 1) * P, :], ob[:])

    nc.compile()
    return nc


_cache = {}


def _get_nc(N, D):
    key = (N, D)
    if key not in _cache:
        _cache[key] = build_kernel(N, D)
    return _cache[key]


def _run(tgt, trace=False):
    """tgt: [B, N, D] f32. Returns (out [B, N, D] f32, exec_time_ns|None)."""
    tgt = np.ascontiguousarray(np.asarray(tgt, dtype=np.float32))
    B, N, D = tgt.shape
    nc = _get_nc(N, D)
    in_maps = [{"tgt": tgt[b]} for b in range(B)]
    res = run_bass_kernel_spmd(nc, in_maps, core_ids=list(range(B)), trace=trace)
    outp = np.stack([res.results[b]["out"] for b in range(B)], axis=0)
    return outp.astype(np.float32), res.exec_time_ns


def kernel(tgt, query_pos=None, objects_num=None, **_unused):
    out, _ = _run(tgt, trace=False)
    return out
